# revision 13
# baseline (speedup 1.0000x reference)
"""TRN2 Bass kernel for nn_DQNBPP_90838558310651 (GraphAttentionEncoder, n_heads=1).

Strategy: data-parallel over batch across 8 NeuronCores (8 items each).
All activations kept in transposed [feature, token] layout on-chip so every
matmul has its contraction dim on partitions. Scores are computed transposed
([key, query]) which makes the attention mask a per-partition bias folded
into the exp activation, and makes softmax normalization a rank-1 ones-matmul
plus a broadcast multiply -- no on-chip transposes at all.
All matmuls run in float32r (TF32-like, 1 cycle/row at free-dim >= 256).
"""
import sys

sys.path.insert(0, "/opt/trn_rl_repo")

import numpy as np

import concourse.bacc as bacc
import concourse.mybir as mybir
import concourse.tile as tile
from concourse.bass_utils import run_bass_kernel_spmd

B, G, D, L, F = 64, 1024, 256, 2, 1024
NCORES = 8
PB = B // NCORES          # items per core
NORM = 1.0 / np.sqrt(D).item()
NEGBIG = -30000.0         # exp(NEGBIG + x) == 0 in fp32 for any realistic x

P = 128                   # partitions
KT = D // P               # 2 contraction tiles over d
GT = G // P               # 8 tiles over keys
FT = F // P               # 8 tiles over ffn hidden
QC = 512                  # token chunk (matmul moving free dim)
NQC = G // QC             # 2

f32 = mybir.dt.float32
f32r = mybir.dt.float32r
Exp = mybir.ActivationFunctionType.Exp
Relu = mybir.ActivationFunctionType.Relu
Identity = mybir.ActivationFunctionType.Identity
AX = mybir.AxisListType.X
MULT = mybir.AluOpType.mult
ADD = mybir.AluOpType.add

_cache = {}


def _build(pb=PB, bench_loop=False, reps=1):
    import contextlib

    nc = bacc.Bacc(None, target_bir_lowering=False, debug=False)

    if bench_loop:
        nrt = nc.dram_tensor("nreps", [1, 1], mybir.dt.int32, kind="ExternalInput")
    xT = nc.dram_tensor("xT", [pb, D, G], f32r, kind="ExternalInput")
    mb = nc.dram_tensor("mb", [pb, GT, P], f32, kind="ExternalInput")  # -30000*mask
    Wq = nc.dram_tensor("Wq", [L, D, D], f32r, kind="ExternalInput")
    Wk = nc.dram_tensor("Wk", [L, D, D], f32r, kind="ExternalInput")
    Wv = nc.dram_tensor("Wv", [L, D, D], f32r, kind="ExternalInput")
    Wo = nc.dram_tensor("Wo", [L, D, D], f32r, kind="ExternalInput")
    bo = nc.dram_tensor("bo", [L, D], f32, kind="ExternalInput")
    Wf1 = nc.dram_tensor("Wf1", [L, D, F], f32r, kind="ExternalInput")
    bf1 = nc.dram_tensor("bf1", [L, F], f32, kind="ExternalInput")
    Wf2 = nc.dram_tensor("Wf2", [L, F, D], f32r, kind="ExternalInput")
    bf2 = nc.dram_tensor("bf2", [L, D], f32, kind="ExternalInput")
    outT = nc.dram_tensor("outT", [pb, D, G], f32r, kind="ExternalOutput")
    outM = nc.dram_tensor("outM", [pb, D], f32, kind="ExternalOutput")

    with tile.TileContext(nc) as tc:
        with tc.tile_pool(name="wgt", bufs=1) as wp, \
             tc.tile_pool(name="act", bufs=2) as ap, \
             tc.tile_pool(name="sm", bufs=3) as sp, \
             tc.tile_pool(name="psA", bufs=3, space="PSUM") as psA, \
             tc.tile_pool(name="psB", bufs=4, space="PSUM") as psB, \
             tc.tile_pool(name="psD", bufs=1, space="PSUM") as psD:

            # ---- load weights (resident) ----
            # W* stored [in, out]; SBUF layout [ki, ko, out] so lhsT slices are
            # [:, ko, m0:m1].
            wq_sb, wk_sb, wv_sb, wo_sb, wf1_sb, wf2_sb = [], [], [], [], [], []
            bo_sb, bf1_sb, bf2_sb = [], [], []
            for l in range(L):
                for (nm, dst, src, cols) in (("wq", wq_sb, Wq, D),
                                             ("wk", wk_sb, Wk, D),
                                             ("wv", wv_sb, Wv, D),
                                             ("wo", wo_sb, Wo, D),
                                             ("wf1", wf1_sb, Wf1, F)):
                    t = wp.tile([P, KT, cols], f32r, tag=f"{nm}{l}", name=f"{nm}{l}")
                    nc.sync.dma_start(
                        out=t[:],
                        in_=src[l].rearrange("(ko ki) m -> ki ko m", ki=P))
                    dst.append(t)
                t = wp.tile([P, FT, D], f32r, tag=f"wf2{l}", name=f"wf2{l}")
                nc.sync.dma_start(
                    out=t[:], in_=Wf2[l].rearrange("(fo ki) m -> ki fo m", ki=P))
                wf2_sb.append(t)
                for (nm, dst, src, n) in (("bo", bo_sb, bo, KT),
                                          ("bf1", bf1_sb, bf1, FT),
                                          ("bf2", bf2_sb, bf2, KT)):
                    t = wp.tile([P, n], f32, tag=f"{nm}{l}", name=f"{nm}{l}")
                    nc.sync.dma_start(
                        out=t[:], in_=src[l].rearrange("(o ki) -> ki o", ki=P))
                    dst.append(t)
            ones_f = wp.tile([P, 1], f32)
            ones = wp.tile([P, 1], f32r)
            nc.vector.memset(ones_f[:], 1.0)
            nc.scalar.copy(ones[:], ones_f[:])

            if bench_loop:
                nt = sp.tile([1, 1], mybir.dt.int32, tag="nrt", name="nrt")
                nc.sync.dma_start(out=nt[:], in_=nrt[:])
                nv = nc.values_load(nt[0:1, 0:1], min_val=0, max_val=1 << 20)
                ET = mybir.EngineType
                loop_cm = tc.For_i(
                    0, nv, 1,
                    hint_engines=(ET.PE, ET.Activation, ET.DVE, ET.SP, ET.Pool))
            else:
                loop_cm = contextlib.nullcontext()
            with loop_cm:
                env = locals()
                for _rep in range(reps):
                    _emit_items(nc, tc, pb, env)

    nc.compile()
    return nc


def _emit_items(nc, tc, pb, env):
    (xT, mb, outT, outM, ap, sp, psA, psB, psD,
     wq_sb, wk_sb, wv_sb, wo_sb, wf1_sb, wf2_sb,
     bo_sb, bf1_sb, bf2_sb, ones) = (
        env["xT"], env["mb"], env["outT"], env["outM"], env["ap"], env["sp"],
        env["psA"], env["psB"], env["psD"],
        env["wq_sb"], env["wk_sb"], env["wv_sb"], env["wo_sb"],
        env["wf1_sb"], env["wf2_sb"],
        env["bo_sb"], env["bf1_sb"], env["bf2_sb"], env["ones"])
    if True:
            for it in range(pb):
                # ---- load item ----
                h = [ap.tile([P, G], f32r, tag=f"h{k}", name=f"h{k}") for k in range(KT)]
                for k in range(KT):
                    nc.sync.dma_start(out=h[k][:], in_=xT[it, k * P:(k + 1) * P, :])
                mbt = sp.tile([P, GT], f32, tag="mbt", name="mbt")
                nc.sync.dma_start(
                    out=mbt[:], in_=mb[it].rearrange("gt ki -> ki gt"))

                for l in range(L):
                    # ---- q/k projections (transposed layout) ----
                    qT = [ap.tile([P, G], f32r, tag=f"qT{m}", name=f"qT{m}") for m in range(KT)]
                    kT = [ap.tile([P, G], f32r, tag=f"kT{m}", name=f"kT{m}") for m in range(KT)]
                    for (w_sb, dst) in ((wq_sb, qT), (wk_sb, kT)):
                        for m in range(KT):
                            for qc in range(NQC):
                                ps = psA.tile([P, QC], f32, tag="mm", name="mm")
                                for k in range(KT):
                                    nc.tensor.matmul(
                                        ps[:], w_sb[l][:, k, m * P:(m + 1) * P],
                                        h[k][:, qc * QC:(qc + 1) * QC],
                                        start=(k == 0), stop=(k == KT - 1))
                                nc.scalar.copy(
                                    dst[m][:, qc * QC:(qc + 1) * QC], ps[:])

                    heads = [ap.tile([P, G], f32r, tag=f"hd{m}", name=f"hd{m}") for m in range(KT)]
                    vN = ap.tile([P, GT, D], f32r, tag="vN", name="vN")
                    ets, esums, recips, rbs = [], [], [], []

                    def scores_chunk(qc):
                        et = ap.tile([P, GT, QC], f32r, tag="et", name="et")
                        ets.append(et)
                        for gt in range(GT):
                            ps = psA.tile([P, QC], f32, tag="mm", name="mm")
                            for k in range(KT):
                                nc.tensor.matmul(
                                    ps[:], kT[k][:, gt * P:(gt + 1) * P],
                                    qT[k][:, qc * QC:(qc + 1) * QC],
                                    start=(k == 0), stop=(k == KT - 1))
                            nc.scalar.activation(
                                et[:, gt, :], ps[:], Exp,
                                bias=mbt[:, gt:gt + 1], scale=NORM)

                    def esum_tree(qc):
                        lvl = [ets[qc][:, gt, :] for gt in range(GT)]
                        while len(lvl) > 1:
                            nxt = []
                            for i in range(0, len(lvl), 2):
                                t = sp.tile([P, QC], f32r, tag="esum",
                                            name="esum", bufs=8)
                                nc.vector.tensor_tensor(out=t[:], in0=lvl[i],
                                                        in1=lvl[i + 1], op=ADD)
                                nxt.append(t[:])
                            lvl = nxt
                        esums.append(lvl[0])

                    def heads_mms(qc, m):
                        ps = psB.tile([P, QC], f32, tag="acc", name="acc")
                        for gt in range(GT):
                            nc.tensor.matmul(
                                ps[:], vN[:, gt, m * P:(m + 1) * P],
                                ets[qc][:, gt, :],
                                start=(gt == 0), stop=(gt == GT - 1))
                        return ps

                    def den_recip_rb(qc):
                        den = psD.tile([1, QC], f32, tag="den", name="den")
                        nc.tensor.matmul(den[:], ones[:], esums[qc],
                                         start=True, stop=True)
                        recip = sp.tile([1, QC], f32, tag="recip", name="recip")
                        nc.vector.reciprocal(recip[:], den[:])
                        rb = sp.tile([P, QC], f32, tag="rb", name="rb")
                        nc.gpsimd.partition_broadcast(rb[:], recip[:])
                        rbs.append(rb)

                    def heads_evict(qc, m, ps):
                        nc.scalar.copy(
                            heads[m][:, qc * QC:(qc + 1) * QC], ps[:])

                    def wo_group(m, qc):
                        ps = psA.tile([P, QC], f32, tag="mm", name="mm")
                        for k in range(KT):
                            nc.tensor.matmul(
                                ps[:], wo_sb[l][:, k, m * P:(m + 1) * P],
                                heads[k][:, qc * QC:(qc + 1) * QC],
                                start=(k == 0), stop=(k == KT - 1))
                        sl = h[m][:, qc * QC:(qc + 1) * QC]
                        tmp = sp.tile([P, QC], f32, tag="wtmp", name="wtmp",
                                      bufs=3)
                        nc.vector.tensor_tensor(out=tmp[:], in0=ps[:],
                                                in1=rbs[qc][:], op=MULT)
                        nc.vector.tensor_tensor(out=sl, in0=sl, in1=tmp[:], op=ADD)
                        nc.vector.tensor_scalar(
                            out=sl, in0=sl, scalar1=bo_sb[l][:, m:m + 1],
                            scalar2=None, op0=ADD)

                    # emission order tuned for the in-order PE stream
                    scores_chunk(0)
                    # v projection, overlaps ch0 exps on ACT
                    for gt in range(GT):
                        ps = psA.tile([P, D], f32, tag="mm", name="mm")
                        for k in range(KT):
                            nc.tensor.matmul(
                                ps[:], h[k][:, gt * P:(gt + 1) * P],
                                wv_sb[l][:, k, :],
                                start=(k == 0), stop=(k == KT - 1))
                        nc.scalar.copy(vN[:, gt, :], ps[:])
                    scores_chunk(1)
                    esum_tree(0)
                    ps00 = heads_mms(0, 0)
                    ps01 = heads_mms(0, 1)
                    den_recip_rb(0)
                    esum_tree(1)
                    ps10 = heads_mms(1, 0)
                    heads_evict(0, 0, ps00)
                    heads_evict(0, 1, ps01)
                    ps11 = heads_mms(1, 1)
                    den_recip_rb(1)
                    wo_group(0, 0)
                    heads_evict(1, 0, ps10)
                    heads_evict(1, 1, ps11)
                    wo_group(1, 0)
                    wo_group(0, 1)
                    wo_group(1, 1)

                    # ---- feed-forward + residual (f1 staged 3 ahead of f2) ----
                    for gc in range(NQC):
                        acc = [psB.tile([P, QC], f32, tag="acc", name="acc") for _ in range(KT)]
                        fts = {}

                        def f1_group(ft):
                            ps = psA.tile([P, QC], f32, tag="mm", name="mm")
                            for k in range(KT):
                                nc.tensor.matmul(
                                    ps[:], wf1_sb[l][:, k, ft * P:(ft + 1) * P],
                                    h[k][:, gc * QC:(gc + 1) * QC],
                                    start=(k == 0), stop=(k == KT - 1))
                            fT = sp.tile([P, QC], f32r, tag="fT", name="fT",
                                         bufs=5)
                            nc.scalar.activation(fT[:], ps[:], Relu,
                                                 bias=bf1_sb[l][:, ft:ft + 1])
                            fts[ft] = fT

                        def f2_group(ft):
                            for m in range(KT):
                                nc.tensor.matmul(
                                    acc[m][:], wf2_sb[l][:, ft, m * P:(m + 1) * P],
                                    fts[ft][:],
                                    start=(ft == 0), stop=(ft == FT - 1))

                        for ft in range(3):
                            f1_group(ft)
                        for ft in range(FT):
                            f2_group(ft)
                            if ft + 3 < FT:
                                f1_group(ft + 3)
                        for m in range(KT):
                            sl = h[m][:, gc * QC:(gc + 1) * QC]
                            nc.vector.tensor_tensor(out=sl, in0=sl,
                                                    in1=acc[m][:], op=ADD)
                            nc.vector.tensor_scalar(
                                out=sl, in0=sl, scalar1=bf2_sb[l][:, m:m + 1],
                                scalar2=None, op0=ADD)

                # ---- store outputs ----
                for k in range(KT):
                    nc.sync.dma_start(out=outT[it, k * P:(k + 1) * P, :], in_=h[k][:])
                    s = sp.tile([P, 1], f32, tag="mean", name="mean")
                    nc.vector.reduce_sum(s[:], h[k][:], axis=AX)
                    nc.vector.tensor_scalar_mul(s[:], s[:], 1.0 / G)
                    nc.sync.dma_start(out=outM[it, k * P:(k + 1) * P], in_=s[:])


def _in_maps(x, mask, Wq, Wk, Wv, Wo, bo, Wf1, bf1, Wf2, bf2):
    xT = np.ascontiguousarray(
        np.transpose(np.asarray(x, np.float32).reshape(NCORES, PB, G, D),
                     (0, 1, 3, 2)))
    mb = (np.asarray(mask).reshape(NCORES, PB, GT, P).astype(np.float32) * NEGBIG)
    shared = {nm: np.ascontiguousarray(v, np.float32)
              for nm, v in (("Wq", Wq), ("Wk", Wk), ("Wv", Wv), ("Wo", Wo),
                            ("bo", bo), ("Wf1", Wf1), ("bf1", bf1),
                            ("Wf2", Wf2), ("bf2", bf2))}
    return [{"xT": xT[c], "mb": mb[c], **shared} for c in range(NCORES)]


def _postprocess(results):
    hT = np.stack([r["outT"] for r in results])       # [NCORES, PB, D, G]
    h = np.ascontiguousarray(
        np.transpose(hT, (0, 1, 3, 2))).reshape(B, G, D).astype(np.float32)
    hm = np.stack([r["outM"] for r in results]).reshape(B, D).astype(np.float32)
    return h, hm


def kernel(x, mask, Wq, Wk, Wv, Wo, bo, Wf1, bf1, Wf2, bf2):
    if "nc" not in _cache:
        _cache["nc"] = _build(PB)
    in_maps = _in_maps(x, mask, Wq, Wk, Wv, Wo, bo, Wf1, bf1, Wf2, bf2)
    res = run_bass_kernel_spmd(_cache["nc"], in_maps, list(range(NCORES))).results
    return _postprocess(res)


# revision 22
# speedup vs baseline: 1.0497x; 1.0497x over previous
"""TRN2 Bass kernel for nn_DQNBPP_90838558310651 (GraphAttentionEncoder, n_heads=1).

Strategy: data-parallel over batch across 8 NeuronCores (8 items each).
All activations kept in transposed [feature, token] layout on-chip so every
matmul has its contraction dim on partitions. Scores are computed transposed
([key, query]) which makes the attention mask a per-partition bias folded
into the exp activation, and makes softmax normalization a rank-1 ones-matmul
plus a broadcast multiply -- no on-chip transposes at all.
All matmuls run in float32r (TF32-like, 1 cycle/row at free-dim >= 256).
"""
import sys

sys.path.insert(0, "/opt/trn_rl_repo")

import numpy as np

import concourse.bacc as bacc
import concourse.mybir as mybir
import concourse.tile as tile
from concourse.bass_utils import run_bass_kernel_spmd

B, G, D, L, F = 64, 1024, 256, 2, 1024
NCORES = 8
PB = B // NCORES          # items per core
NORM = 1.0 / np.sqrt(D).item()
NEGBIG = -30000.0         # exp(NEGBIG + x) == 0 in fp32 for any realistic x

P = 128                   # partitions
KT = D // P               # 2 contraction tiles over d
GT = G // P               # 8 tiles over keys
FT = F // P               # 8 tiles over ffn hidden
QC = 512                  # token chunk (matmul moving free dim)
NQC = G // QC             # 2

f32 = mybir.dt.float32
f32r = mybir.dt.float32r
Exp = mybir.ActivationFunctionType.Exp
Relu = mybir.ActivationFunctionType.Relu
Identity = mybir.ActivationFunctionType.Identity
AX = mybir.AxisListType.X
MULT = mybir.AluOpType.mult
ADD = mybir.AluOpType.add
MAXOP = mybir.AluOpType.max

_cache = {}


def _build(pb=PB, bench_loop=False, reps=1, phases=("qkv", "attn", "ffn")):
    import contextlib

    nc = bacc.Bacc(None, target_bir_lowering=False, debug=False)

    if bench_loop:
        nrt = nc.dram_tensor("nreps", [1, 1], mybir.dt.int32, kind="ExternalInput")
    xT = nc.dram_tensor("xT", [pb, D, G], f32r, kind="ExternalInput")
    mb = nc.dram_tensor("mb", [pb, GT, P], f32, kind="ExternalInput")  # -30000*mask
    Wq = nc.dram_tensor("Wq", [L, D, D], f32r, kind="ExternalInput")
    Wk = nc.dram_tensor("Wk", [L, D, D], f32r, kind="ExternalInput")
    Wv = nc.dram_tensor("Wv", [L, D, D], f32r, kind="ExternalInput")
    Wo = nc.dram_tensor("Wo", [L, D, D], f32r, kind="ExternalInput")
    bo = nc.dram_tensor("bo", [L, D], f32, kind="ExternalInput")
    Wf1 = nc.dram_tensor("Wf1", [L, D, F], f32r, kind="ExternalInput")
    bf1 = nc.dram_tensor("bf1", [L, F], f32, kind="ExternalInput")
    Wf2 = nc.dram_tensor("Wf2", [L, F, D], f32r, kind="ExternalInput")
    bf2 = nc.dram_tensor("bf2", [L, D], f32, kind="ExternalInput")
    outT = nc.dram_tensor("outT", [pb, D, G], f32r, kind="ExternalOutput")
    outM = nc.dram_tensor("outM", [pb, D], f32, kind="ExternalOutput")

    with tile.TileContext(nc) as tc:
        with tc.tile_pool(name="wgt", bufs=1) as wp, \
             tc.tile_pool(name="act", bufs=2) as ap, \
             tc.tile_pool(name="sm", bufs=3) as sp, \
             tc.tile_pool(name="psA", bufs=4, space="PSUM") as psA, \
             tc.tile_pool(name="psB", bufs=3, space="PSUM") as psB, \
             tc.tile_pool(name="psD", bufs=1, space="PSUM") as psD:

            # ---- load weights (resident) ----
            # W* stored [in, out]; SBUF layout [ki, ko, out] so lhsT slices are
            # [:, ko, m0:m1].
            wq_sb, wk_sb, wv_sb, wo_sb, wf1_sb, wf2_sb = [], [], [], [], [], []
            bo_sb, bf1_sb, bf2_sb = [], [], []
            for l in range(L):
                for (nm, dst, src, cols) in (("wq", wq_sb, Wq, D),
                                             ("wk", wk_sb, Wk, D),
                                             ("wv", wv_sb, Wv, D),
                                             ("wo", wo_sb, Wo, D),
                                             ("wf1", wf1_sb, Wf1, F)):
                    t = wp.tile([P, KT, cols], f32r, tag=f"{nm}{l}", name=f"{nm}{l}")
                    nc.sync.dma_start(
                        out=t[:],
                        in_=src[l].rearrange("(ko ki) m -> ki ko m", ki=P))
                    dst.append(t)
                t = wp.tile([P, FT, D], f32r, tag=f"wf2{l}", name=f"wf2{l}")
                nc.sync.dma_start(
                    out=t[:], in_=Wf2[l].rearrange("(fo ki) m -> ki fo m", ki=P))
                wf2_sb.append(t)
                for (nm, dst, src, n) in (("bo", bo_sb, bo, KT),
                                          ("bf1", bf1_sb, bf1, FT),
                                          ("bf2", bf2_sb, bf2, KT)):
                    t = wp.tile([P, n], f32, tag=f"{nm}{l}", name=f"{nm}{l}")
                    nc.sync.dma_start(
                        out=t[:], in_=src[l].rearrange("(o ki) -> ki o", ki=P))
                    dst.append(t)
            ones_f = wp.tile([P, 1], f32)
            ones = wp.tile([P, 1], f32r)
            nc.vector.memset(ones_f[:], 1.0)
            nc.scalar.copy(ones[:], ones_f[:])

            if bench_loop:
                nt = sp.tile([1, 1], mybir.dt.int32, tag="nrt", name="nrt")
                nc.sync.dma_start(out=nt[:], in_=nrt[:])
                nv = nc.values_load(nt[0:1, 0:1], min_val=0, max_val=1 << 20)
                ET = mybir.EngineType
                loop_cm = tc.For_i(
                    0, nv, 1,
                    hint_engines=(ET.PE, ET.Activation, ET.DVE, ET.SP, ET.Pool))
            else:
                loop_cm = contextlib.nullcontext()
            with loop_cm:
                env = locals()
                for _rep in range(reps):
                    _emit_items(nc, tc, pb, env, phases)

    nc.compile()
    return nc


def _emit_items(nc, tc, pb, env, phases=("qkv", "attn", "ffn")):
    (xT, mb, outT, outM, ap, sp, psA, psB, psD,
     wq_sb, wk_sb, wv_sb, wo_sb, wf1_sb, wf2_sb,
     bo_sb, bf1_sb, bf2_sb, ones) = (
        env["xT"], env["mb"], env["outT"], env["outM"], env["ap"], env["sp"],
        env["psA"], env["psB"], env["psD"],
        env["wq_sb"], env["wk_sb"], env["wv_sb"], env["wo_sb"],
        env["wf1_sb"], env["wf2_sb"],
        env["bo_sb"], env["bf1_sb"], env["bf2_sb"], env["ones"])
    if True:
            def _item(it):
                # ---- load item ----
                h = [ap.tile([P, G], f32r, tag=f"h{k}", name=f"h{k}") for k in range(KT)]
                for k in range(KT):
                    nc.sync.dma_start(out=h[k][:], in_=xT[it, k * P:(k + 1) * P, :])
                mbt = sp.tile([P, GT], f32, tag="mbt", name="mbt")
                nc.sync.dma_start(
                    out=mbt[:], in_=mb[it].rearrange("gt ki -> ki gt"))

                for l in range(L):
                    if "qkv" not in phases:
                        break
                    # ---- q/k projections (transposed layout) ----
                    qT = [ap.tile([P, G], f32r, tag=f"qT{m}", name=f"qT{m}") for m in range(KT)]
                    kT = [ap.tile([P, G], f32r, tag=f"kT{m}", name=f"kT{m}") for m in range(KT)]
                    for (w_sb, dst) in ((wq_sb, qT), (wk_sb, kT)):
                        for m in range(KT):
                            for qc in range(NQC):
                                ps = psA.tile([P, QC], f32, tag="mm", name="mm")
                                for k in range(KT):
                                    nc.tensor.matmul(
                                        ps[:], w_sb[l][:, k, m * P:(m + 1) * P],
                                        h[k][:, qc * QC:(qc + 1) * QC],
                                        start=(k == 0), stop=(k == KT - 1))
                                nc.vector.tensor_copy(
                                    dst[m][:, qc * QC:(qc + 1) * QC], ps[:])

                    if "attn" not in phases:
                        continue
                    heads = [ap.tile([P, G], f32r, tag=f"hd{m}", name=f"hd{m}") for m in range(KT)]
                    vN = ap.tile([P, GT, D], f32r, tag="vN", name="vN")
                    ets, esums, recips, rbs = [], [], [], []

                    def scores_chunk(qc):
                        et = ap.tile([P, GT, QC], f32r, tag="et", name="et")
                        ets.append(et)
                        for gt in range(GT):
                            ps = psA.tile([P, QC], f32, tag="mm", name="mm")
                            for k in range(KT):
                                nc.tensor.matmul(
                                    ps[:], kT[k][:, gt * P:(gt + 1) * P],
                                    qT[k][:, qc * QC:(qc + 1) * QC],
                                    start=(k == 0), stop=(k == KT - 1))
                            nc.scalar.activation(
                                et[:, gt, :], ps[:], Exp,
                                bias=mbt[:, gt:gt + 1], scale=NORM)

                    def esum_tree(qc):
                        lvl = [ets[qc][:, gt, :] for gt in range(GT)]
                        while len(lvl) > 1:
                            nxt = []
                            for i in range(0, len(lvl), 2):
                                t = sp.tile([P, QC], f32r, tag="esum",
                                            name="esum", bufs=8)
                                nc.vector.tensor_tensor(out=t[:], in0=lvl[i],
                                                        in1=lvl[i + 1], op=ADD)
                                nxt.append(t[:])
                            lvl = nxt
                        esums.append(lvl[0])

                    def heads_mms(qc, m):
                        ps = psB.tile([P, QC], f32, tag="acc", name="acc")
                        for gt in range(GT):
                            nc.tensor.matmul(
                                ps[:], vN[:, gt, m * P:(m + 1) * P],
                                ets[qc][:, gt, :],
                                start=(gt == 0), stop=(gt == GT - 1))
                        return ps

                    def den_recip_rb(qc):
                        den = psD.tile([1, QC], f32, tag="den", name="den")
                        nc.tensor.matmul(den[:], ones[:], esums[qc],
                                         start=True, stop=True)
                        recip = sp.tile([1, QC], f32, tag="recip", name="recip")
                        nc.vector.reciprocal(recip[:], den[:])
                        rb = sp.tile([P, QC], f32, tag="rb", name="rb")
                        nc.gpsimd.partition_broadcast(rb[:], recip[:])
                        rbs.append(rb)

                    def heads_evict(qc, m, ps):
                        nc.scalar.copy(
                            heads[m][:, qc * QC:(qc + 1) * QC], ps[:])

                    def wo_group(m, qc):
                        ps = psA.tile([P, QC], f32, tag="mm", name="mm")
                        for k in range(KT):
                            nc.tensor.matmul(
                                ps[:], wo_sb[l][:, k, m * P:(m + 1) * P],
                                heads[k][:, qc * QC:(qc + 1) * QC],
                                start=(k == 0), stop=(k == KT - 1))
                        sl = h[m][:, qc * QC:(qc + 1) * QC]
                        tmp = sp.tile([P, QC], f32, tag="wtmp", name="wtmp",
                                      bufs=3)
                        nc.vector.tensor_tensor(out=tmp[:], in0=ps[:],
                                                in1=rbs[qc][:], op=MULT)
                        nc.vector.tensor_tensor(out=sl, in0=sl, in1=tmp[:], op=ADD)
                        nc.vector.tensor_scalar(
                            out=sl, in0=sl, scalar1=bo_sb[l][:, m:m + 1],
                            scalar2=None, op0=ADD)

                    # emission order tuned for the in-order PE stream
                    scores_chunk(0)
                    # v projection, overlaps ch0 exps on ACT
                    for gt in range(GT):
                        ps = psA.tile([P, D], f32, tag="mm", name="mm")
                        for k in range(KT):
                            nc.tensor.matmul(
                                ps[:], h[k][:, gt * P:(gt + 1) * P],
                                wv_sb[l][:, k, :],
                                start=(k == 0), stop=(k == KT - 1))
                        nc.scalar.copy(vN[:, gt, :], ps[:])
                    scores_chunk(1)
                    esum_tree(0)
                    ps00 = heads_mms(0, 0)
                    ps01 = heads_mms(0, 1)
                    den_recip_rb(0)
                    esum_tree(1)
                    ps10 = heads_mms(1, 0)
                    heads_evict(0, 0, ps00)
                    heads_evict(0, 1, ps01)
                    ps11 = heads_mms(1, 1)
                    den_recip_rb(1)
                    wo_group(0, 0)
                    heads_evict(1, 0, ps10)
                    heads_evict(1, 1, ps11)
                    wo_group(1, 0)
                    wo_group(0, 1)
                    wo_group(1, 1)

                    yield "attn_done"
                    if "ffn" not in phases:
                        continue
                    # ---- feed-forward + residual (f1 staged 3 ahead of f2) ----
                    for gc in range(NQC):
                        acc = [psB.tile([P, QC], f32, tag="acc", name="acc") for _ in range(KT)]
                        fts = {}

                        def f1_group(ft):
                            ps = psA.tile([P, QC], f32, tag="mm", name="mm")
                            for k in range(KT):
                                nc.tensor.matmul(
                                    ps[:], wf1_sb[l][:, k, ft * P:(ft + 1) * P],
                                    h[k][:, gc * QC:(gc + 1) * QC],
                                    start=(k == 0), stop=(k == KT - 1))
                            fT = sp.tile([P, QC], f32r, tag="fT", name="fT",
                                         bufs=5)
                            if ft % 2 == 0:
                                nc.scalar.activation(fT[:], ps[:], Relu,
                                                     bias=bf1_sb[l][:, ft:ft + 1])
                            else:
                                nc.vector.tensor_scalar(
                                    out=fT[:], in0=ps[:],
                                    scalar1=bf1_sb[l][:, ft:ft + 1],
                                    scalar2=0.0, op0=ADD, op1=MAXOP)
                            fts[ft] = fT

                        def f2_group(ft):
                            for m in range(KT):
                                nc.tensor.matmul(
                                    acc[m][:], wf2_sb[l][:, ft, m * P:(m + 1) * P],
                                    fts[ft][:],
                                    start=(ft == 0), stop=(ft == FT - 1))

                        for ft in range(4):
                            f1_group(ft)
                        for ft in range(FT):
                            f2_group(ft)
                            if ft + 4 < FT:
                                f1_group(ft + 4)
                        for m in range(KT):
                            sl = h[m][:, gc * QC:(gc + 1) * QC]
                            nc.vector.tensor_tensor(out=sl, in0=sl,
                                                    in1=acc[m][:], op=ADD)
                            nc.vector.tensor_scalar(
                                out=sl, in0=sl, scalar1=bf2_sb[l][:, m:m + 1],
                                scalar2=None, op0=ADD)
                    yield "ffn_done"

                # ---- store outputs ----
                for k in range(KT):
                    nc.sync.dma_start(out=outT[it, k * P:(k + 1) * P, :], in_=h[k][:])
                    s = sp.tile([P, 1], f32, tag="mean", name="mean")
                    nc.vector.reduce_sum(s[:], h[k][:], axis=AX)
                    nc.vector.tensor_scalar_mul(s[:], s[:], 1.0 / G)
                    nc.sync.dma_start(out=outM[it, k * P:(k + 1) * P], in_=s[:])

            for p0 in range(0, pb, 2):
                pair = [_item(i) for i in range(p0, min(p0 + 2, pb))]
                alive = True
                while alive:
                    alive = False
                    for g in pair:
                        if next(g, "done") != "done":
                            alive = True


def _in_maps(x, mask, Wq, Wk, Wv, Wo, bo, Wf1, bf1, Wf2, bf2):
    xT = np.ascontiguousarray(
        np.transpose(np.asarray(x, np.float32).reshape(NCORES, PB, G, D),
                     (0, 1, 3, 2)))
    mb = (np.asarray(mask).reshape(NCORES, PB, GT, P).astype(np.float32) * NEGBIG)
    shared = {nm: np.ascontiguousarray(v, np.float32)
              for nm, v in (("Wq", Wq), ("Wk", Wk), ("Wv", Wv), ("Wo", Wo),
                            ("bo", bo), ("Wf1", Wf1), ("bf1", bf1),
                            ("Wf2", Wf2), ("bf2", bf2))}
    return [{"xT": xT[c], "mb": mb[c], **shared} for c in range(NCORES)]


def _postprocess(results):
    hT = np.stack([r["outT"] for r in results])       # [NCORES, PB, D, G]
    h = np.ascontiguousarray(
        np.transpose(hT, (0, 1, 3, 2))).reshape(B, G, D).astype(np.float32)
    hm = np.stack([r["outM"] for r in results]).reshape(B, D).astype(np.float32)
    return h, hm


def kernel(x, mask, Wq, Wk, Wv, Wo, bo, Wf1, bf1, Wf2, bf2):
    if "nc" not in _cache:
        _cache["nc"] = _build(PB)
    in_maps = _in_maps(x, mask, Wq, Wk, Wv, Wo, bo, Wf1, bf1, Wf2, bf2)
    res = run_bass_kernel_spmd(_cache["nc"], in_maps, list(range(NCORES))).results
    return _postprocess(res)
